# revision 1
# baseline (speedup 1.0000x reference)
"""Cross-attention kernel for 8 Trainium2 NeuronCores.

Reference computation (per batch element a):
  K = X @ Wk, Q = L @ Wq, V = X @ Wv          (each head uses a full 256-dim slice)
  S_i = Q_i @ K_i^T / sqrt(32); P = softmax(S); A_i = P_i @ V_i
  out = concat_i(A_i) @ Wu + bu

Sharding: core c = 2*a + hg handles batch a and head-group hg (4 heads, 1024
projection columns). The final head-concat matmul is split over head-groups;
the two partial outputs per batch element are summed on the host (the
"all-reduce after unify_heads"), which also adds the bias.

All matmuls run as float32r (fp32 storage; PE processes 1 row/cycle at
N>=256 vs 4 cycles/row for plain fp32, with identical numerics on TRN2 --
both use the same reduced-precision PE path, measured ~1.6e-4 max rel err).

Device layouts (per core) -- everything keeps the contraction dim on SBUF
partitions so no on-device transposes are needed:
  XT, LT           [256, 1024]  x^T / latent^T   (host pre-transposes)
  WK, WQ, WV       [256, 1024]  natural
  WU               [1024, 256]  natural
  KT = (X@WK)^T    [1024(n), 1024(s)]  via lhsT=WK-tile, rhs=XT
  QT = (L@WQ)^T    [1024(n), 1024(y)]  via lhsT=WQ-tile, rhs=LT
  V  = X@WV        [1024(s), 1024(n)]  via lhsT=XT-tile, rhs=WV
  S^T_i            [b, y] psum via lhsT=KT-tile, rhs=QT        (per head i)
  P^T_i = exp(.)   [b, y] sbuf, ACT exp with scale fused
  sums_i           [1, y] via lhsT=ones column  (softmax denominators)
  A^T_i            [c, y] psum via lhsT=V-tile, rhs=P^T; normalized by
                   1/sums (broadcast via K=1 ones matmul) on PSUM->SBUF copy
  O               [y, e] via lhsT=A^T-tile, rhs=WU-tile
"""

import math
import sys

import numpy as np

sys.path.insert(0, "/opt/trn_rl_repo")

import concourse.bass as bass  # noqa: E402
import concourse.mybir as mybir  # noqa: E402
from concourse import bacc, bass_isa  # noqa: E402
from concourse.bass_utils import run_bass_kernel_spmd  # noqa: E402
from concourse.tile import TileContext  # noqa: E402

F32 = mybir.dt.float32
F32R = mybir.dt.float32r
EXP = mybir.ActivationFunctionType.Exp

B, S, E = 4, 1024, 256          # batch, seq, embed
HEADS = 8                        # total heads; each head dim = E (source quirk)
N_CORES = 8
HG_HEADS = 4                     # heads per head-group (per core)
NH = HG_HEADS * E                # projection columns per core = 1024
SCALE = 1.0 / math.sqrt(E // HEADS)   # 1/sqrt(32)

P = 128                          # SBUF partitions
NT = NH // P                     # 8 partition tiles of the projection dim
ST = S // P                      # 8 partition tiles of the seq dim
NCH = 512                        # matmul moving-dim chunk
SCH = S // NCH                   # 2 chunks of 512 over seq

_CACHE = {}

import os as _os
SCRATCH_BUFS = int(_os.environ.get("K_SCRATCH", "10"))
SC_BUFS = int(_os.environ.get("K_SC", "4"))
PA_BUFS = int(_os.environ.get("K_PA", "3"))
MISC_BUFS = int(_os.environ.get("K_MISC", "2"))


def _build():
    nc = bacc.Bacc(target_bir_lowering=False)

    XT = nc.dram_tensor("XT", [E, S], F32R, kind="ExternalInput")
    LT = nc.dram_tensor("LT", [E, S], F32R, kind="ExternalInput")
    WK = nc.dram_tensor("WK", [E, NH], F32R, kind="ExternalInput")
    WQ = nc.dram_tensor("WQ", [E, NH], F32R, kind="ExternalInput")
    WV = nc.dram_tensor("WV", [E, NH], F32R, kind="ExternalInput")
    WU = nc.dram_tensor("WU", [NH, E], F32R, kind="ExternalInput")
    O = nc.dram_tensor("O", [S, E], F32, kind="ExternalOutput")

    ET = E // P  # 2 partition tiles of the embed (contraction) dim

    with TileContext(nc) as tc:
        with tc.tile_pool(name="persist", bufs=1) as pp, \
             tc.tile_pool(name="scratch", bufs=SCRATCH_BUFS) as sp, \
             tc.tile_pool(name="small", bufs=2) as mp, \
             tc.tile_pool(name="psum", bufs=1, space="PSUM") as ps:

            # ---- phase 0: load inputs (chunked so compute starts early) ----
            def alloc_in(nm):
                return [sp.tile([P, S], F32R, tag="big", name=f"{nm}{t}")
                        for t in range(ET)]

            xt, wk, wv, lt, wq = (alloc_in(n) for n in ("xt", "wk", "wv", "lt", "wq"))

            def dma_chunk(tiles, dram, e, c):
                nc.sync.dma_start(
                    out=tiles[e][:, c * NCH:(c + 1) * NCH],
                    in_=dram[e * P:(e + 1) * P, c * NCH:(c + 1) * NCH])

            # issue order: operands of the first KT groups first
            for e in range(ET):
                nc.sync.dma_start(out=wk[e][:, 0:P],
                                  in_=WK[e * P:(e + 1) * P, 0:P])
            for e in range(ET):
                dma_chunk(xt, XT, e, 0)
            for e in range(ET):
                nc.sync.dma_start(out=wk[e][:, P:NCH],
                                  in_=WK[e * P:(e + 1) * P, P:NCH])
            for e in range(ET):
                dma_chunk(wk, WK, e, 1)
            for e in range(ET):
                dma_chunk(xt, XT, e, 1)
            # lt/wq/wv go through the SWDGE (gpsimd) queue: its descriptor
            # generation runs in parallel with the HWDGE queue handling the
            # fill-critical xt/wk stream
            for c in range(SCH):
                for e in range(ET):
                    dma_chunk(wv, WV, e, c)
            for c in range(SCH):
                for e in range(ET):
                    dma_chunk(lt, LT, e, c)
            for c in range(SCH):
                for e in range(ET):
                    dma_chunk(wq, WQ, e, c)
            wu_all = pp.tile([P, NT * E], F32R, tag="wu", name="wu_all")
            nc.sync.dma_start(out=wu_all[:].rearrange("p (t e) -> p t e", t=NT),
                              in_=WU.rearrange("(t p) e -> p t e", p=P))
            wu = [wu_all[:, t * E:(t + 1) * E] for t in range(NT)]

            # ---- phase 1: projections KT, QT (transposed), V (natural) ----
            kt, qt, v = [], [], []
            for nt in range(NT):
                kt.append(pp.tile([P, S], F32R, tag=f"kt{nt}", name=f"kt{nt}"))
                qt.append(pp.tile([P, S], F32R, tag=f"qt{nt}", name=f"qt{nt}"))
                v.append(pp.tile([P, S], F32R, tag=f"v{nt}", name=f"v{nt}"))

            # alternate PSUM->SBUF evictions between DVE and ACT so neither
            # engine gates the PE during the projection phase
            evict_ctr = [0]

            def evict(dst_ap, src_ap):
                evict_ctr[0] += 1
                if evict_ctr[0] % 2 == 0:
                    nc.vector.tensor_copy(dst_ap, src_ap)
                else:
                    nc.scalar.activation(dst_ap, src_ap,
                                         mybir.ActivationFunctionType.Copy)

            def proj(dst, lhs_tiles, lhs_cols, rhs_tiles, nt, c, nm):
                sl = bass.ts(c, NCH)
                pk = ps.tile([P, NCH], F32, tag="pA", bufs=PA_BUFS, name=f"p{nm}{nt}{c}")
                for e in range(ET):
                    nc.tensor.matmul(pk[:], lhs_tiles[e][:, lhs_cols],
                                     rhs_tiles[e][:, sl],
                                     start=(e == 0), stop=(e == ET - 1))
                evict(dst[nt][:, sl], pk[:])

            for c in range(SCH):
                for nt in range(NT):
                    proj(kt, wk, slice(nt * P, (nt + 1) * P), xt, nt, c, "k")
            for c in range(SCH):
                for nt in range(NT):
                    proj(v, xt, slice(nt * P, (nt + 1) * P), wv, nt, c, "v")
            for c in range(SCH):
                for nt in range(NT):
                    proj(qt, wq, slice(nt * P, (nt + 1) * P), lt, nt, c, "q")

            # ---- phase 2: attention per head ----
            at = []
            for nt in range(NT):
                at.append(pp.tile([P, S], F32R, tag=f"at{nt}", name=f"at{nt}"))

            o_acc = []

            pt_h = {}
            acc_h = {}
            acc2_h = {}
            sums_h = {}
            rec_h = {}

            def st_group(h, c, bt):
                c0 = 2 * h
                sl = bass.ts(c, NCH)
                pt, acc = pt_h[h], acc_h[h]
                pss = ps.tile([P, NCH], F32, tag="sc", bufs=SC_BUFS,
                              name=f"pss{h}{bt}{c}")
                for cj in range(2):
                    nc.tensor.matmul(pss[:], kt[c0 + cj][:, bt * P:(bt + 1) * P],
                                     qt[c0 + cj][:, sl],
                                     start=(cj == 0), stop=(cj == 1))
                nc.scalar.activation(pt[bt][:, sl], pss[:], EXP, scale=SCALE)
                # denominator accumulation pipelined with the exps; the
                # two half-trees run on DVE and the (otherwise idle) GPSIMD
                acc2 = acc2_h[h]
                if bt == 1:
                    nc.vector.tensor_add(acc[:, sl], pt[0][:, sl], pt[1][:, sl])
                elif bt in (2, 3):
                    nc.vector.tensor_add(acc[:, sl], acc[:, sl], pt[bt][:, sl])
                elif bt == 5:
                    nc.gpsimd.tensor_add(acc2[:, sl], pt[4][:, sl], pt[5][:, sl])
                elif bt in (6, 7):
                    nc.gpsimd.tensor_add(acc2[:, sl], acc2[:, sl], pt[bt][:, sl])

            def sums_chain(h, c):
                # softmax denominators: single GPSIMD all-reduce over the
                # partition dim (broadcasting the sum to all partitions),
                # then invert on DVE
                sl = bass.ts(c, NCH)
                nc.vector.tensor_add(acc_h[h][:, sl], acc_h[h][:, sl],
                                     acc2_h[h][:, sl])
                nc.gpsimd.partition_all_reduce(
                    rec_h[h][:, sl], acc_h[h][:, sl].bitcast(F32),
                    channels=P, reduce_op=bass_isa.ReduceOp.add)
                nc.vector.reciprocal(rec_h[h][:, sl], rec_h[h][:, sl])

            def at_group(h, c, ct):
                # A^T accumulation over b; normalize on PSUM->SBUF eviction
                sl = bass.ts(c, NCH)
                vsl = slice(h * E + ct * P, h * E + (ct + 1) * P)
                pa = ps.tile([P, NCH], F32, tag="pA", bufs=PA_BUFS,
                             name=f"pa{h}{ct}{c}")
                for bt in range(ST):
                    nc.tensor.matmul(pa[:], v[bt][:, vsl], pt_h[h][bt][:, sl],
                                     start=(bt == 0), stop=(bt == ST - 1))
                nc.vector.tensor_mul(at[2 * h + ct][:, sl], pa[:],
                                     rec_h[h][:, sl])

            def head_alloc(h):
                pt_h[h] = [sp.tile([P, S], F32R, tag="big", name=f"pt{h}{bt}")
                           for bt in range(ST)]
                acc_h[h] = mp.tile([P, S], F32R, tag="sacc", name=f"sacc{h}")
                acc2_h[h] = mp.tile([P, S], F32R, tag="sacc2", name=f"sacc2{h}")
                rec_h[h] = mp.tile([P, S], F32, tag="rec", bufs=2,
                                   name=f"rec{h}")

            def out_tile(yt):
                po = ps.tile([P, E], F32, tag="sc", bufs=SC_BUFS,
                             name=f"po{yt}")
                for ht in range(NT):
                    nc.tensor.matmul(po[:], at[ht][:, yt * P:(yt + 1) * P],
                                     wu[ht],
                                     start=(ht == 0), stop=(ht == NT - 1))
                osb = mp.tile([P, E], F32, tag="osb", bufs=6, name=f"osb{yt}")
                evict(osb[:], po[:])
                nc.sync.dma_start(out=O[yt * P:(yt + 1) * P, :], in_=osb[:])

            # cross-head software pipeline: head h's chunk-1 A^T groups are
            # woven into head h+1's chunk-0 score stream, so the PE always has
            # matmul work while ACT drains the exp queue
            for h in range(HG_HEADS):
                head_alloc(h)
                for bt in range(ST):
                    st_group(h, 0, bt)
                    if h > 0:
                        if bt == 2:
                            at_group(h - 1, 1, 0)
                        elif bt == 5:
                            at_group(h - 1, 1, 1)
                sums_chain(h, 0)
                for bt in range(ST):
                    st_group(h, 1, bt)
                    if bt == 2:
                        at_group(h, 0, 0)
                    elif bt == 5:
                        at_group(h, 0, 1)
                sums_chain(h, 1)
            LAST = HG_HEADS - 1

            # ---- phase 3: output tiles woven with head-3's final A^T ----
            out_tile(0)
            out_tile(1)
            at_group(LAST, 1, 0)
            out_tile(2)
            out_tile(3)
            at_group(LAST, 1, 1)
            for yt in range(4, ST):
                out_tile(yt)

    nc.compile()
    return nc


def kernel(batch, latent, Wk, Wq, Wv, Wu, bu):
    batch = np.asarray(batch, dtype=np.float32)
    latent = np.asarray(latent, dtype=np.float32)
    Wk = np.asarray(Wk, dtype=np.float32)
    Wq = np.asarray(Wq, dtype=np.float32)
    Wv = np.asarray(Wv, dtype=np.float32)
    Wu = np.asarray(Wu, dtype=np.float32)
    bu = np.asarray(bu, dtype=np.float32)

    if "nc" not in _CACHE:
        _CACHE["nc"] = _build()
    nc = _CACHE["nc"]

    in_maps = []
    for core in range(N_CORES):
        a, hg = core // 2, core % 2
        cols = slice(hg * NH, (hg + 1) * NH)
        in_maps.append({
            "XT": np.ascontiguousarray(batch[a].T),
            "LT": np.ascontiguousarray(latent[a].T),
            "WK": np.ascontiguousarray(Wk[:, cols]),
            "WQ": np.ascontiguousarray(Wq[:, cols]),
            "WV": np.ascontiguousarray(Wv[:, cols]),
            "WU": np.ascontiguousarray(Wu[cols, :]),
        })

    _CACHE["in_maps"] = in_maps
    res = run_bass_kernel_spmd(nc, in_maps, core_ids=list(range(N_CORES)))

    out = np.empty((B, S, E), dtype=np.float32)
    for a in range(B):
        out[a] = res.results[2 * a]["O"] + res.results[2 * a + 1]["O"] + bu
    return out



# revision 7
# speedup vs baseline: 2.2220x; 2.2220x over previous
"""Cross-attention kernel for 8 Trainium2 NeuronCores.

Reference computation (per batch element a, head i, full 256-dim per head):
  K_i = X @ Wk_i, Q_i = L @ Wq_i, V_i = X @ Wv_i
  S_i = Q_i @ K_i^T / sqrt(32); P = softmax(S); A_i = P_i @ V_i
  out = sum_i A_i @ Wu_i + bu

Host-side weight fusion (weights only, exact):
  M_i  = Wq_i @ Wk_i^T   =>  S_i = (L @ M_i) @ X^T     (K projection gone)
  W~_i = Wv_i @ Wu_i     =>  out = sum_i P_i @ (X @ W~_i)   (Wu matmul gone)

Sharding: core c = 2*a + hg handles batch a and head-group hg (4 heads).
The two partial outputs per batch element are summed on the host, which
also adds the bias.

Device data flow (per core, everything fp32r, contraction dim on SBUF
partitions so no on-device transposes):
  GT = (L @ MC)^T   [1024(g), 1024(y)]   via lhsT=MC-tile, rhs=LT
  U  = X @ WT       [1024(b), 4*(256+1)] via lhsT=XT-tile, rhs=WT, evicted
                    with a stride that leaves a ones column after each head
  S^T_i [b,y] psum  via lhsT=XT-tile, rhs=GT(head i rows)
  P^T_i = exp(.)    [b,y] sbuf, ACT exp with 1/sqrt(32) scale fused
  out[y, 257] psum  += P^T_i-tile^T @ [U_i | 1]  over b-tiles; col 256 is
                    the softmax denominator (ones-column trick), already in
                    [y-partition] orientation
  eviction          rec = 1/psum[:,256] (DVE, [128,1]); fused
                    osb = psum[:, :256] * rec + osb   (scalar_tensor_tensor)
  O [1024, 256]     natural row-major output, DMA'd per y-tile

The ones-column trick removes the entire softmax-denominator reduction
(adder tree + partition_all_reduce + wide reciprocals) from the baseline,
which was stalling the PE ~11us per head-chunk.
"""

import math
import sys

import numpy as np

sys.path.insert(0, "/opt/trn_rl_repo")

import concourse.bass as bass  # noqa: E402
import concourse.mybir as mybir  # noqa: E402
from concourse import bacc  # noqa: E402
from concourse.bass_utils import run_bass_kernel_spmd  # noqa: E402
from concourse.tile import TileContext  # noqa: E402

F32 = mybir.dt.float32
F32R = mybir.dt.float32r
EXP = mybir.ActivationFunctionType.Exp
COPY = mybir.ActivationFunctionType.Copy
MULT = mybir.AluOpType.mult
ADD = mybir.AluOpType.add

B, S, E = 4, 1024, 256          # batch, seq, embed
HEADS = 8                        # total heads; each head dim = E (source quirk)
N_CORES = 8
HG = 4                           # heads per head-group (per core)
NH = HG * E                      # fused-weight columns per core = 1024
SCALE = 1.0 / math.sqrt(E // HEADS)   # 1/sqrt(32)

P = 128                          # SBUF partitions
ET = E // P                      # 2 contraction tiles over embed
ST = S // P                      # 8 tiles over seq (b or y)
NCH = 512                        # matmul moving-dim chunk
EC = E + 2                       # 258: out + sums col + pad (fp32r needs even N)
UW = HG * EC                     # 1032: U tile width (4 heads + sums/pad cols)

_CACHE = {}


def _build():
    nc = bacc.Bacc(target_bir_lowering=False)

    XT = nc.dram_tensor("XT", [E, S], F32R, kind="ExternalInput")
    LT = nc.dram_tensor("LT", [E, S], F32R, kind="ExternalInput")
    MC = nc.dram_tensor("MC", [E, NH], F32R, kind="ExternalInput")
    WT = nc.dram_tensor("WT", [E, NH], F32R, kind="ExternalInput")
    O = nc.dram_tensor("O", [S, E], F32, kind="ExternalOutput")

    with TileContext(nc) as tc:
        with tc.tile_pool(name="persist", bufs=1) as pp, \
             tc.tile_pool(name="psum", bufs=1, space="PSUM") as ps:

            xt = [pp.tile([P, S], F32R, tag=f"xt{e}", name=f"xt{e}") for e in range(ET)]
            lt = [pp.tile([P, S], F32R, tag=f"lt{e}", name=f"lt{e}") for e in range(ET)]
            mc = [pp.tile([P, S], F32R, tag=f"mc{e}", name=f"mc{e}") for e in range(ET)]
            wt = [pp.tile([P, S], F32R, tag=f"wt{e}", name=f"wt{e}") for e in range(ET)]
            gt = [pp.tile([P, S], F32R, tag=f"gt{g}", name=f"gt{g}") for g in range(ST)]
            u = [pp.tile([P, UW], F32R, tag=f"u{bt}", name=f"u{bt}") for bt in range(ST)]
            # two ping-pong sets of P^T tiles (head h uses set h%2)
            pt = [[pp.tile([P, S], F32R, tag=f"pt{s}_{bt}", name=f"pt{s}_{bt}")
                   for bt in range(ST)] for s in range(2)]
            osb = [pp.tile([P, E], F32, tag=f"osb{yt}", name=f"osb{yt}")
                   for yt in range(ST)]
            rc = [pp.tile([P, ST], F32, tag=f"rc{s}", name=f"rc{s}") for s in range(2)]

            # ---- input DMA, ordered so compute can start early ----
            def dma_in(dst_tiles, dram, e, c0, c1):
                nc.sync.dma_start(out=dst_tiles[e][:, c0:c1],
                                  in_=dram[e * P:(e + 1) * P, c0:c1])

            for e in range(ET):
                dma_in(mc, MC, e, 0, NCH)       # GT g0-3 lhsT
            for e in range(ET):
                dma_in(lt, LT, e, 0, NCH)       # scores chunk-0 rhs source
            for e in range(ET):
                dma_in(mc, MC, e, NCH, S)       # GT g4-7 lhsT
            for e in range(ET):
                dma_in(lt, LT, e, NCH, S)
            for c in range(2):
                for e in range(ET):
                    dma_in(xt, XT, e, c * NCH, (c + 1) * NCH)
            for e in range(ET):
                dma_in(wt, WT, e, 0, S)

            # ones columns of the U tiles (col 256 of each head's 257 block)
            for bt in range(ST):
                for h in range(HG):
                    nc.gpsimd.memset(
                        u[bt][:, h * EC + E:(h + 1) * EC].bitcast(F32), 1.0)

            # alternate PSUM->SBUF evictions between DVE and ACT
            # (GPSIMD cannot access PSUM on TRN2)
            ev_ctr = [0]

            def evict(dst_ap, src_ap):
                ev_ctr[0] += 1
                if ev_ctr[0] % 2 == 0:
                    nc.vector.tensor_copy(dst_ap, src_ap)
                else:
                    nc.scalar.activation(dst_ap, src_ap, COPY)

            # ---- GT projection: GT[g,y] = sum_e MC[e,g] * LT[e,y] ----
            def gt_proj(g, c):
                sl = bass.ts(c, NCH)
                pg = ps.tile([P, NCH], F32, tag="sc", bufs=4, name=f"pg{g}{c}")
                for e in range(ET):
                    nc.tensor.matmul(pg[:], mc[e][:, g * P:(g + 1) * P],
                                     lt[e][:, sl],
                                     start=(e == 0), stop=(e == ET - 1))
                evict(gt[g][:, sl], pg[:])

            # ---- U projection: U[b,g] = sum_e XT[e,b] * WT[e,g], strided
            #      eviction leaves the ones column after each head's block ----
            def u_proj(bt, gc):
                pu = ps.tile([P, NCH], F32, tag="sc", bufs=4, name=f"pu{bt}{gc}")
                for e in range(ET):
                    nc.tensor.matmul(pu[:], xt[e][:, bt * P:(bt + 1) * P],
                                     wt[e][:, gc * NCH:(gc + 1) * NCH],
                                     start=(e == 0), stop=(e == ET - 1))
                uv = u[bt][:].rearrange("p (h x) -> p h x", h=HG)
                pv = pu[:].rearrange("p (h x) -> p h x", h=2)
                evict(uv[:, 2 * gc:2 * gc + 2, 0:E], pv[:])

            # ---- scores + exp for head h, chunk c ----
            def scores(h, c):
                sl = bass.ts(c, NCH)
                pts = pt[h % 2]
                for bt in range(ST):
                    pss = ps.tile([P, NCH], F32, tag="sc", bufs=4,
                                  name=f"pss{h}{c}{bt}")
                    for e in range(ET):
                        nc.tensor.matmul(pss[:], xt[e][:, bt * P:(bt + 1) * P],
                                         gt[2 * h + e][:, sl],
                                         start=(e == 0), stop=(e == ET - 1))
                    nc.scalar.activation(pts[bt][:, sl], pss[:], EXP, scale=SCALE)

            # ---- out accumulation for head h, y-tiles of chunk c;
            #      psum col 256 = softmax denominator; fused normalize ----
            def outq(h, c):
                pts = pt[h % 2]
                r = rc[h % 2]
                for yt in range(4 * c, 4 * c + 4):
                    po = ps.tile([P, EC], F32, tag="po", bufs=4,
                                 name=f"po{h}{yt}")
                    for bt in range(ST):
                        nc.tensor.matmul(po[:], pts[bt][:, yt * P:(yt + 1) * P],
                                         u[bt][:, h * EC:(h + 1) * EC],
                                         start=(bt == 0), stop=(bt == ST - 1))
                    nc.vector.reciprocal(r[:, yt:yt + 1], po[:, E:E + 1])
                    if h == 0:
                        nc.vector.tensor_scalar_mul(osb[yt][:], po[:, 0:E],
                                                    r[:, yt:yt + 1])
                    else:
                        nc.vector.scalar_tensor_tensor(osb[yt][:], po[:, 0:E],
                                                       r[:, yt:yt + 1],
                                                       osb[yt][:], MULT, ADD)
                    if h == HG - 1:
                        nc.sync.dma_start(out=O[yt * P:(yt + 1) * P, :],
                                          in_=osb[yt][:])

            # ---- PE program order: keep the PE busy while ACT drains exps ----
            for c in range(2):
                for g in range(ST):
                    gt_proj(g, c)
            scores(0, 0)
            scores(0, 1)
            for bt in range(ST):
                for gc in range(2):
                    u_proj(bt, gc)
            scores(1, 0)
            scores(1, 1)
            outq(0, 0)
            outq(0, 1)
            scores(2, 0)
            scores(2, 1)
            outq(1, 0)
            outq(1, 1)
            scores(3, 0)
            scores(3, 1)
            outq(2, 0)
            outq(2, 1)
            outq(3, 0)
            outq(3, 1)

    nc.compile()
    return nc


def kernel(batch, latent, Wk, Wq, Wv, Wu, bu):
    batch = np.asarray(batch, dtype=np.float32)
    latent = np.asarray(latent, dtype=np.float32)
    Wk = np.asarray(Wk, dtype=np.float32)
    Wq = np.asarray(Wq, dtype=np.float32)
    Wv = np.asarray(Wv, dtype=np.float32)
    Wu = np.asarray(Wu, dtype=np.float32)
    bu = np.asarray(bu, dtype=np.float32)

    if "nc" not in _CACHE:
        _CACHE["nc"] = _build()
    nc = _CACHE["nc"]

    in_maps = []
    for core in range(N_CORES):
        a, hg = core // 2, core % 2
        mcs, wts = [], []
        for j in range(HG):
            i = hg * HG + j                      # global head index
            cols = slice(i * E, (i + 1) * E)
            mcs.append(Wq[:, cols] @ Wk[:, cols].T)
            wts.append(Wv[:, cols] @ Wu[cols.start:cols.stop, :])
        in_maps.append({
            "XT": np.ascontiguousarray(batch[a].T),
            "LT": np.ascontiguousarray(latent[a].T),
            "MC": np.ascontiguousarray(np.concatenate(mcs, axis=1)),
            "WT": np.ascontiguousarray(np.concatenate(wts, axis=1)),
        })

    _CACHE["in_maps"] = in_maps
    res = run_bass_kernel_spmd(nc, in_maps, core_ids=list(range(N_CORES)))

    out = np.empty((B, S, E), dtype=np.float32)
    for a in range(B):
        out[a] = res.results[2 * a]["O"] + res.results[2 * a + 1]["O"] + bu
    return out
